# revision 1
# baseline (speedup 1.0000x reference)
"""GATClassifier (2x GATConv + mean-pool + linear) on 8 Trainium2 NeuronCores.

v2: bf16 datapath + gather-based a_dst routing + chunked AllGathers.

Sharding: nodes range-partitioned 6250/core (padded 6272 = 49*128); each core
owns edges whose destination lands in its shard. Table rows are 384 bf16 cols
(768 B): [h 0:256 | a_src 256:260 | a_dst 260:264 | pad]. The table is stored
half-major ("chunk-major"): half A = windows 0..23 of every core, half B =
windows 24..48, so each half can be AllGathered as soon as its windows are
built (overlapping the collective with remaining compute).

Per window (128 dst nodes): dma_gather the incident edges' source rows from
the AllGathered halves (768 B/edge), plus a second 256 B/edge gather of the
dst nodes' a_dst block from the core's LOCAL table half (tokens < 3200, so no
int16-range split needed).  e = a_src(gathered) + a_dst(gathered);
exp(leaky(e)) via the abs trick; one batched one-hot build (iota==dstloc,
broadcast tensor_tensor) and one batched msg multiply; per-block bf16 matmuls
accumulate numerator+denominator in f32 PSUM.  Layer-2 table rows are built
in layer-1's window sink (elu -> transpose -> matmul) so layer-2's chunked
AllGathers also overlap layer-1's window loop.

Everything is SPMD-uniform: all core-specific info arrives as data.
"""

import math
import os

import numpy as np

# ---------------------------------------------------------------- constants
N = 50000       # nodes
E = 800000      # directed edges before self loops
IN = 128        # in channels
H = 4           # heads
C = 64          # channels per head
HC = H * C      # 256
G = 64          # graphs
NC_ = 8         # cores
P = 128
SH = N // NC_           # 6250 real nodes per shard
NW = math.ceil(SH / P)  # 49 windows per core
SHP = NW * P            # 6272 padded shard rows
NWA = 24                # windows in half A
NWB = NW - NWA          # 25 windows in half B
SHA = NWA * P           # 3072 local rows in half A
SHB = NWB * P           # 3200
NHA = NC_ * SHA         # 24576 gathered rows half A (< 32768 for int16)
NHB = NC_ * SHB         # 25600
RW = 384                # bf16 cols per table row (768 B)
XC = HC + 2 * H         # 264 meaningful f32 cols of [h|a_src|a_dst]
EC = HC + 4 * H         # 272 bf16 cols written per row: h bf16 + 8 f32 e-vals


def _bf16():
    import ml_dtypes
    return ml_dtypes.bfloat16


def _wrap16(tok: np.ndarray) -> np.ndarray:
    """dma_gather index layout: token i lives at [i%16, i//16], replicated
    into all 8 groups of 16 partitions."""
    assert tok.size % 16 == 0
    w = tok.reshape(-1, 16).T.astype(np.int16)  # [16, L/16]
    return np.tile(w, (8, 1))                   # [128, L/16]


def _preprocess(edge_index: np.ndarray, batch: np.ndarray):
    """Host-side integer-only preprocessing: shard edges by dst, sort into
    (window, src-half, src) order, pad to 16-token blocks, and emit per-core
    index/dstloc/ad-token arrays plus static per-window block counts."""
    src = np.concatenate([edge_index[0], np.arange(N, dtype=np.int64)])
    dst = np.concatenate([edge_index[1], np.arange(N, dtype=np.int64)])
    cs, ls = src // SH, src % SH
    ws, ps = ls // P, ls % P
    half = (ws >= NWA).astype(np.int64)
    tok = np.where(half == 0, cs * SHA + ws * P + ps,
                   cs * SHB + (ws - NWA) * P + ps)

    owner = dst // SH
    dl = dst % SH
    wd, dloc = dl // P, dl % P

    counts = np.zeros((NC_, NW, 2), dtype=np.int64)
    per_core = []
    for c in range(NC_):
        m = owner == c
        t_, h_, w_, d_ = tok[m], half[m], wd[m], dloc[m]
        order = np.lexsort((t_, h_, w_))
        t_, h_, w_, d_ = t_[order], h_[order], w_[order], d_[order]
        np.add.at(counts[c], (w_, h_), 1)
        per_core.append((t_, h_, w_, d_))

    maxcnt = counts.max(axis=0)                       # [NW, 2]
    ntlo = np.maximum(16, (maxcnt[:, 0] + 15) // 16 * 16)
    nthi = np.maximum(16, (maxcnt[:, 1] + 15) // 16 * 16)
    blo = (ntlo + P - 1) // P
    bhi = (nthi + P - 1) // P
    bw = blo + bhi
    totb = int(bw.sum())
    bwmax = int(bw.max())
    gb0 = np.concatenate([[0], np.cumsum(bw)[:-1]]).astype(np.int64)

    dstloc = np.full((NC_, P, totb), -1.0, dtype=np.float32)
    adtok = np.zeros((NC_, P, totb * 8), dtype=np.int16)
    ilo_l, ihi_l = [], []
    for c in range(NC_):
        t_, h_, w_, d_ = per_core[c]
        lo_parts, hi_parts = [], []
        for wi in range(NW):
            base = wi * P if wi < NWA else (wi - NWA) * P
            ad_stream = np.zeros(int(bw[wi]) * P, dtype=np.int64)
            for hf, nt, bcnt in ((0, int(ntlo[wi]), int(blo[wi])),
                                 (1, int(nthi[wi]), int(bhi[wi]))):
                m = (w_ == wi) & (h_ == hf)
                nreal = int(m.sum())
                tt = np.zeros(nt, dtype=np.int64)
                tt[:nreal] = t_[m]
                dd = np.full(bcnt * P, -1.0, dtype=np.float32)
                dd[:nreal] = d_[m]
                (lo_parts if hf == 0 else hi_parts).append(tt)
                b0 = int(gb0[wi]) + (int(blo[wi]) if hf else 0)
                dstloc[c, :, b0:b0 + bcnt] = dd.reshape(bcnt, P).T
                s0 = int(blo[wi]) * P if hf else 0
                ad_stream[s0:s0 + bcnt * P] = (
                    base + np.maximum(dd, 0.0).astype(np.int64))
            adtok[c, :, int(gb0[wi]) * 8:(int(gb0[wi]) + int(bw[wi])) * 8] = (
                _wrap16(ad_stream))
        ilo_l.append(np.concatenate([_wrap16(x) for x in lo_parts], axis=1))
        ihi_l.append(np.concatenate([_wrap16(x) for x in hi_parts], axis=1))
    ilo = np.stack(ilo_l)
    ihi = np.stack(ihi_l)
    lo_off8 = np.concatenate([[0], np.cumsum(ntlo // 16)[:-1]]).astype(np.int64)
    hi_off8 = np.concatenate([[0], np.cumsum(nthi // 16)[:-1]]).astype(np.int64)

    # batch (graph id) per local node slot; -1 on ghost slots
    batchloc = np.full((NC_, P, NW), -1.0, dtype=np.float32)
    for c in range(NC_):
        b = np.full(SHP, -1.0, dtype=np.float32)
        b[:SH] = batch[c * SH:(c + 1) * SH].astype(np.float32)
        batchloc[c] = b.reshape(NW, P).T

    return dict(
        blo=blo.astype(int), bhi=bhi.astype(int), bw=bw.astype(int),
        ntlo=ntlo.astype(int), nthi=nthi.astype(int),
        gb0=gb0, totb=totb, bwmax=bwmax,
        ilo=ilo, ihi=ihi, adtok=adtok, lo_off8=lo_off8, hi_off8=hi_off8,
        dstloc=dstloc, batchloc=batchloc,
    )


def _fold(Wm, a_s, a_d, b):
    """[W | A_src | A_dst] columns and matching extended bias."""
    K = Wm.shape[0]
    As = np.einsum("khc,hc->kh", Wm.reshape(K, H, C), a_s)
    Ad = np.einsum("khc,hc->kh", Wm.reshape(K, H, C), a_d)
    WR = np.concatenate([Wm, As, Ad], axis=1).astype(np.float32)   # [K, 264]
    be = np.concatenate(
        [b, np.einsum("hc,hc->h", b.reshape(H, C), a_s),
         np.einsum("hc,hc->h", b.reshape(H, C), a_d)]
    ).astype(np.float32)                                           # [264]
    return WR, be


def _build(meta):
    import concourse.bacc as bacc
    import concourse.mybir as mybir
    import concourse.tile as tile

    stage = int(os.environ.get("KSTAGE", "5"))
    reps = int(os.environ.get("KREPS", "1"))
    ksub = int(os.environ.get("KSUB", "2"))
    kgl = int(os.environ.get("KGL", "1"))    # do lo/hi gathers
    kga = int(os.environ.get("KGA", "1"))    # do ad gather
    ksink = int(os.environ.get("KSINK", "1"))  # run sinks
    kagmid = int(os.environ.get("KAGMID", "1"))  # emit AG2 mid-loop
    kbg = int(os.environ.get("KBG", "4"))      # gather pool bufs
    kbn = int(os.environ.get("KBN", "3"))      # psum numerator bufs
    ksp = int(os.environ.get("KSP", "0"))      # single_packet gathers
    kq = int(os.environ.get("KQ", "4"))        # swdge queues
    ked = int(os.environ.get("KED", "1"))      # 1: ad dma_gather, 0: PE route

    f32 = mybir.dt.float32
    bf = mybir.dt.bfloat16
    i16 = mybir.dt.int16
    Act = mybir.ActivationFunctionType
    Alu = mybir.AluOpType

    blo, bw, gb0 = meta["blo"], meta["bw"], meta["gb0"]
    ntlo, nthi = meta["ntlo"], meta["nthi"]
    lo_off8, hi_off8 = meta["lo_off8"], meta["hi_off8"]
    TOTB, BWMAX = meta["totb"], meta["bwmax"]
    NLO8, NHI8 = int((ntlo // 16).sum()), int((nthi // 16).sum())

    nc = bacc.Bacc("TRN2", target_bir_lowering=False, debug=False,
                   num_devices=NC_, num_swdge_queues=kq)

    # ------------------------------------------------------------- tensors
    xT = nc.dram_tensor("xT", [P, SHP], f32, kind="ExternalInput")
    W1R = nc.dram_tensor("W1R", [IN, XC], f32, kind="ExternalInput")
    W2Ra = nc.dram_tensor("W2Ra", [P, XC], f32, kind="ExternalInput")
    W2Rb = nc.dram_tensor("W2Rb", [P, XC], f32, kind="ExternalInput")
    b1e = nc.dram_tensor("b1e", [P, XC], f32, kind="ExternalInput")
    b2e = nc.dram_tensor("b2e", [P, XC], f32, kind="ExternalInput")
    Wlin = nc.dram_tensor("Wlin", [P, 4], f32, kind="ExternalInput")
    blin = nc.dram_tensor("blin", [G, 2], f32, kind="ExternalInput")
    iota128 = nc.dram_tensor("iota128", [P, P], bf, kind="ExternalInput")
    iota64 = nc.dram_tensor("iota64", [P, G], f32, kind="ExternalInput")
    ident = nc.dram_tensor("ident", [P, P], f32, kind="ExternalInput")
    identB = nc.dram_tensor("identB", [P, P], bf, kind="ExternalInput")
    dstloc = nc.dram_tensor("dstloc", [P, TOTB], bf, kind="ExternalInput")
    idxlo = nc.dram_tensor("idxlo", [P, NLO8], i16, kind="ExternalInput")
    idxhi = nc.dram_tensor("idxhi", [P, NHI8], i16, kind="ExternalInput")
    idxad = nc.dram_tensor("idxad", [P, TOTB * 8], i16, kind="ExternalInput")
    batchloc = nc.dram_tensor("batchloc", [P, NW], f32, kind="ExternalInput")

    logits = nc.dram_tensor("logits", [G, 2], f32, kind="ExternalOutput")

    T1sA = nc.dram_tensor("T1sA", [SHA, RW], bf, kind="Internal")
    T1sB = nc.dram_tensor("T1sB", [SHB, RW], bf, kind="Internal")
    T1h0 = nc.dram_tensor("T1h0", [NHA, RW], bf, kind="Internal",
                          addr_space="Shared")
    T1h1 = nc.dram_tensor("T1h1", [NHB, RW], bf, kind="Internal",
                          addr_space="Shared")
    T2sA = nc.dram_tensor("T2sA", [SHA, RW], bf, kind="Internal")
    T2sB = nc.dram_tensor("T2sB", [SHB, RW], bf, kind="Internal")
    T2h0 = nc.dram_tensor("T2h0", [NHA, RW], bf, kind="Internal",
                          addr_space="Shared")
    T2h1 = nc.dram_tensor("T2h1", [NHB, RW], bf, kind="Internal",
                          addr_space="Shared")
    prd = nc.dram_tensor("prd", [G, HC + 1], f32, kind="Internal")
    prs = nc.dram_tensor("prs", [G, HC + 1], f32, kind="Internal",
                         addr_space="Shared")

    grp = [list(range(NC_))]

    with tile.TileContext(nc) as tc:
        with (
            tc.tile_pool(name="const", bufs=1) as cp,
            tc.tile_pool(name="work", bufs=3) as wp,
            tc.tile_pool(name="gat", bufs=kbg) as gp,
            tc.tile_pool(name="adp", bufs=kbg) as ap_,
            tc.tile_pool(name="sel", bufs=3) as sp,
            tc.tile_pool(name="msg", bufs=3) as mp,
            tc.tile_pool(name="outp", bufs=3) as op_,
            tc.tile_pool(name="ppre", bufs=1, space="PSUM") as ppre,
            tc.tile_pool(name="ptp", bufs=1, space="PSUM") as ptp,
            tc.tile_pool(name="ptb", bufs=2, space="PSUM") as ptb,
            tc.tile_pool(name="ped", bufs=1, space="PSUM") as ped,
            tc.tile_pool(name="pnum", bufs=kbn, space="PSUM") as pnum,
            tc.tile_pool(name="ppool", bufs=1, space="PSUM") as ppl,
        ):
            # ---------------------------------------------------- constants
            def cload(dram, dt):
                tl = cp.tile(list(dram.shape), dt, tag=dram.name)
                nc.sync.dma_start(tl[:], dram[:])
                return tl

            w1r_t = cload(W1R, f32)
            w2a_t = cload(W2Ra, f32)
            w2b_t = cload(W2Rb, f32)
            b1e_t = cload(b1e, f32)
            b2e_t = cload(b2e, f32)
            wl_t = cload(Wlin, f32)
            bl_t = cload(blin, f32)
            io64_t = cload(iota64, f32)
            id_t = cload(ident, f32)
            idB_t = cload(identB, bf)
            ilo_t = cload(idxlo, i16)
            ihi_t = cload(idxhi, i16)
            iad_t = cload(idxad, i16)
            bat_t = cload(batchloc, f32)
            # iota128 as [P, 1, P] and dstloc as [P, TOTB, 1] for the
            # broadcast one-hot build
            io1_t = cp.tile([P, 1, P], bf, tag="io1")
            nc.sync.dma_start(io1_t[:, 0, :], iota128[:])
            dst3_t = cp.tile([P, TOTB, 1], bf, tag="dst3")
            nc.sync.dma_start(
                dst3_t[:].rearrange("p b one -> p (b one)"), dstloc[:])

            # pre-warm gather destination pools: trailing slots of partial
            # gather blocks are read (masked to zero contribution) and must
            # be finite
            for _ in range(kbg):
                gwarm = gp.tile([P, BWMAX, RW], bf, tag="G")
                nc.vector.memset(gwarm[:], 0.0)
                awarm = ap_.tile([P, BWMAX, P], bf, tag="Ad")
                nc.vector.memset(awarm[:], 0.0)

            def trows(TA, TB, w):
                if w < NWA:
                    return TA[w * P:(w + 1) * P, 0:EC]
                return TB[(w - NWA) * P:(w - NWA + 1) * P, 0:EC]

            # --------------------------------------------- layer-1 table
            def phase_a():
                for w in range(NW if stage >= 1 else 0):
                    xt = wp.tile([P, P], f32, tag="xt")
                    nc.sync.dma_start(xt[:], xT[:, w * P:(w + 1) * P])
                    ps = ppre.tile([P, XC], f32, space="PSUM", tag="ppre")
                    nc.tensor.matmul(ps[:], lhsT=xt[:],
                                     rhs=w1r_t[:], start=True, stop=True)
                    h1 = wp.tile([P, EC], bf, tag="h1")
                    nc.vector.tensor_tensor(h1[:, 0:HC], ps[:, 0:HC],
                                            b1e_t[:, 0:HC], op=Alu.add)
                    nc.vector.tensor_tensor(h1[:, HC:EC].bitcast(f32),
                                            ps[:, HC:XC],
                                            b1e_t[:, HC:XC], op=Alu.add)
                    nc.sync.dma_start(trows(T1sA, T1sB, w), h1[:])
                    if stage >= 2 and w == NWA - 1:
                        nc.gpsimd.collective_compute(
                            "AllGather", Alu.bypass, replica_groups=grp,
                            ins=[T1sA[:, :]], outs=[T1h0[:, :]])
                    if stage >= 2 and w == NW - 1:
                        nc.gpsimd.collective_compute(
                            "AllGather", Alu.bypass, replica_groups=grp,
                            ins=[T1sB[:, :]], outs=[T1h1[:, :]])

            # shared window loop -----------------------------------------
            def window_loop(Th0, Th1, TsA, TsB, sink):
                for w in range(NW):
                    BL, BW = int(blo[w]), int(bw[w])
                    b0 = int(gb0[w])
                    wl = w if w < NWA else w - NWA
                    NTL, NTH = int(ntlo[w]), int(nthi[w])
                    Gt = gp.tile([P, BWMAX, RW], bf, tag="G")
                    if kgl:
                        nc.gpsimd.dma_gather(
                            Gt[:, 0:BL, :], Th0[:, :],
                            ilo_t[:, int(lo_off8[w]):
                                  int(lo_off8[w]) + NTL // 16],
                            NTL, NTL, RW, single_packet=bool(ksp))
                        nc.gpsimd.dma_gather(
                            Gt[:, BL:BW, :], Th1[:, :],
                            ihi_t[:, int(hi_off8[w]):
                                  int(hi_off8[w]) + NTH // 16],
                            NTH, NTH, RW, single_packet=bool(ksp),
                            queue_num=1 % kq)
                    TsX = TsA if w < NWA else TsB
                    AdG = ap_.tile([P, BWMAX, P], bf, tag="Ad")
                    if kga and ked:
                        if kq >= 4:
                            BH1 = BW // 2
                            nc.gpsimd.dma_gather(
                                AdG[:, 0:BH1, :], TsX[:, HC:HC + P],
                                iad_t[:, b0 * 8:(b0 + BH1) * 8],
                                BH1 * P, BH1 * P, P, elem_step=RW,
                                single_packet=bool(ksp), queue_num=2)
                            nc.gpsimd.dma_gather(
                                AdG[:, BH1:BW, :], TsX[:, HC:HC + P],
                                iad_t[:, (b0 + BH1) * 8:(b0 + BW) * 8],
                                (BW - BH1) * P, (BW - BH1) * P, P,
                                elem_step=RW,
                                single_packet=bool(ksp), queue_num=3)
                        else:
                            nc.gpsimd.dma_gather(
                                AdG[:, 0:BW, :], TsX[:, HC:HC + P],
                                iad_t[:, b0 * 8:(b0 + BW) * 8],
                                BW * P, BW * P, P, elem_step=RW,
                                single_packet=bool(ksp), queue_num=2 % kq)

                    if ksub == 0:
                        ob = op_.tile([P, HC], f32, tag="ob")
                        nc.vector.tensor_copy(ob[:], Gt[:, 0, 0:HC])
                        if ksink:
                            sink(w, ob)
                        continue

                    # batched one-hot: S[p, b, j] = (j == dstloc[p, b])
                    St = sp.tile([P, BWMAX, P], bf, tag="S")
                    nc.vector.tensor_tensor(
                        St[:, :BW, :],
                        io1_t[:].to_broadcast([P, BW, P]),
                        dst3_t[:, b0:b0 + BW, :].to_broadcast([P, BW, P]),
                        op=Alu.is_equal)

                    # e = a_src(gathered) + a_dst(routed);
                    # exp(leaky(e)) as exp(0.6*(e + (2/3)|e|))
                    ev = mp.tile([P, BWMAX, H], f32, tag="ev")
                    GtF = Gt[:].bitcast(f32)
                    if ked:
                        AdF = AdG[:].bitcast(f32)
                        nc.vector.tensor_tensor(ev[:, :BW, :],
                                                GtF[:, :BW, P:P + H],
                                                AdF[:, :BW, H:2 * H],
                                                op=Alu.add)
                    else:
                        # route a_dst per edge slot via PE: one transpose of
                        # each one-hot block + a tiny matmul against the
                        # window's [128 dst, 4] a_dst values (exact f32)
                        edw = wp.tile([P, 4 * H], bf, tag="edw")
                        nc.sync.dma_start(
                            edw[:],
                            TsX[wl * P:(wl + 1) * P, HC:HC + 4 * H])
                        edwF = edw[:].bitcast(f32)
                        STt = sp.tile([P, BWMAX, P], f32, tag="ST")
                        edp = ped.tile([P, BWMAX, H], f32, space="PSUM",
                                       tag="ed")
                        for b in range(BW):
                            tpb = ptb.tile([P, P], bf, space="PSUM",
                                           tag="tpb")
                            nc.tensor.transpose(tpb[:], St[:, b, :],
                                                idB_t[:])
                            nc.scalar.copy(STt[:, b, :], tpb[:])
                            nc.tensor.matmul(edp[:, b, :],
                                             lhsT=STt[:, b, :],
                                             rhs=edwF[:, H:2 * H],
                                             start=True, stop=True)
                        nc.vector.tensor_tensor(ev[:, :BW, :],
                                                GtF[:, :BW, P:P + H],
                                                edp[:, :BW, :],
                                                op=Alu.add)
                    av = mp.tile([P, BWMAX, H], f32, tag="av")
                    nc.scalar.activation(av[:, :BW, :], ev[:, :BW, :],
                                         Act.Abs, scale=2.0 / 3.0)
                    nc.vector.tensor_tensor(av[:, :BW, :], ev[:, :BW, :],
                                            av[:, :BW, :], op=Alu.add)
                    nc.vector.tensor_scalar(av[:, :BW, :], av[:, :BW, :],
                                            60.0, None, op0=Alu.min)
                    ex = mp.tile([P, BWMAX, H], bf, tag="ex")
                    nc.scalar.activation(ex[:, :BW, :], av[:, :BW, :],
                                         Act.Exp, scale=0.6)

                    # msg = [h * ex | ex]
                    msg = mp.tile([P, BWMAX, XC], bf, tag="msg")
                    nc.vector.tensor_tensor(
                        msg[:, :BW, 0:HC].rearrange(
                            "p b (h c) -> p b h c", h=H),
                        Gt[:, :BW, 0:HC].rearrange(
                            "p b (h c) -> p b h c", h=H),
                        ex[:, :BW, :].to_broadcast([P, BW, H, C]),
                        op=Alu.mult)
                    nc.scalar.copy(msg[:, :BW, HC:HC + H], ex[:, :BW, :])

                    nmp = pnum.tile([P, HC + H], f32, space="PSUM", tag="nm")
                    for b in range(BW):
                        nc.tensor.matmul(nmp[:], lhsT=St[:, b, :],
                                         rhs=msg[:, b, 0:HC + H],
                                         start=(b == 0), stop=(b == BW - 1))
                    rd = mp.tile([P, H], f32, tag="rd")
                    nc.vector.tensor_scalar(rd[:], nmp[:, HC:HC + H],
                                            1e-30, None, op0=Alu.max)
                    nc.vector.reciprocal(rd[:], rd[:])
                    ob = op_.tile([P, HC], f32, tag="ob")
                    nc.vector.tensor_tensor(
                        ob[:].rearrange("p (h c) -> p h c", h=H),
                        nmp[:, 0:HC].rearrange("p (h c) -> p h c", h=H),
                        rd[:].to_broadcast([P, H, C]),
                        op=Alu.mult)
                    if ksink:
                        sink(w, ob)

            # ------------------------- layer-1 sink: fused layer-2 table
            def sink1(w, ob):
                # elu(x) = max(x,0) + exp(min(x,0)) - 1
                t0 = wp.tile([P, HC], f32, tag="elu0")
                nc.vector.tensor_scalar(t0[:], ob[:], 0.0, None, op0=Alu.min)
                nc.scalar.activation(t0[:], t0[:], Act.Exp)
                t1 = wp.tile([P, HC], f32, tag="elu1")
                nc.vector.tensor_scalar(t1[:], ob[:], 0.0, None, op0=Alu.max)
                nc.vector.tensor_tensor(t1[:], t1[:], t0[:], op=Alu.add)
                el = wp.tile([P, HC], f32, tag="el1")
                nc.vector.tensor_scalar(el[:], t1[:], -1.0, None, op0=Alu.add)
                ps2 = ppre.tile([P, XC], f32, space="PSUM", tag="ppre")
                for k in range(2):
                    tp = ptp.tile([P, P], f32, space="PSUM", tag="tp")
                    nc.tensor.transpose(tp[:], el[:, k * P:(k + 1) * P],
                                        id_t[:])
                    et = wp.tile([P, P], f32, tag="eT")
                    nc.scalar.copy(et[:], tp[:])
                    nc.tensor.matmul(ps2[:], lhsT=et[:],
                                     rhs=(w2a_t if k == 0 else w2b_t)[:],
                                     start=(k == 0), stop=(k == 1))
                h2 = wp.tile([P, EC], bf, tag="h2")
                nc.vector.tensor_tensor(h2[:, 0:HC], ps2[:, 0:HC],
                                        b2e_t[:, 0:HC], op=Alu.add)
                nc.vector.tensor_tensor(h2[:, HC:EC].bitcast(f32),
                                        ps2[:, HC:XC],
                                        b2e_t[:, HC:XC], op=Alu.add)
                nc.sync.dma_start(trows(T2sA, T2sB, w), h2[:])
                if stage >= 4 and kagmid and w == NWA - 1:
                    nc.gpsimd.collective_compute(
                        "AllGather", Alu.bypass, replica_groups=grp,
                        ins=[T2sA[:, :]], outs=[T2h0[:, :]])
                if stage >= 4 and w == NW - 1:
                    if not kagmid:
                        nc.gpsimd.collective_compute(
                            "AllGather", Alu.bypass, replica_groups=grp,
                            ins=[T2sA[:, :]], outs=[T2h0[:, :]])
                    nc.gpsimd.collective_compute(
                        "AllGather", Alu.bypass, replica_groups=grp,
                        ins=[T2sB[:, :]], outs=[T2h1[:, :]])

            # -------------------------------- layer-2 sink: mean pooling
            plp = ppl.tile([G, HC + 1], f32, space="PSUM", tag="pool")

            def sink2(w, ob):
                t0 = op_.tile([P, HC], f32, tag="e2a")
                nc.vector.tensor_scalar(t0[:], ob[:], 0.0, None, op0=Alu.min)
                nc.scalar.activation(t0[:], t0[:], Act.Exp)
                t1 = op_.tile([P, HC], f32, tag="e2b")
                nc.vector.tensor_scalar(t1[:], ob[:], 0.0, None, op0=Alu.max)
                nc.vector.tensor_tensor(t1[:], t1[:], t0[:], op=Alu.add)
                el = op_.tile([P, HC + 1], f32, tag="el2")
                nc.vector.tensor_scalar(el[:, 0:HC], t1[:], -1.0, None,
                                        op0=Alu.add)
                nc.vector.memset(el[:, HC:HC + 1], 1.0)
                bm = op_.tile([P, G], f32, tag="bm")
                nc.vector.tensor_scalar(bm[:], io64_t[:],
                                        bat_t[:, w:w + 1], None,
                                        op0=Alu.is_equal)
                nc.tensor.matmul(plp[:], lhsT=bm[:], rhs=el[:],
                                 start=(w == 0), stop=(w == NW - 1))

            def phases_cde():
                if stage >= 3:
                    window_loop(T1h0, T1h1, T1sA, T1sB, sink1)
                if stage >= 5:
                    window_loop(T2h0, T2h1, T2sA, T2sB, sink2)

                    # ----------------------------------------- epilogue
                    pls = wp.tile([G, HC + 1], f32, tag="pls")
                    nc.vector.tensor_copy(pls[:], plp[:])
                    nc.sync.dma_start(prd[:, :], pls[:])
                    nc.gpsimd.collective_compute(
                        "AllReduce", Alu.add, replica_groups=grp,
                        ins=[prd[:, :]], outs=[prs[:, :]])
                    pr = wp.tile([G, HC + 1], f32, tag="pr")
                    nc.sync.dma_start(pr[:], prs[:, :])
                    cnt = wp.tile([G, 1], f32, tag="cnt")
                    nc.vector.tensor_scalar(cnt[:], pr[:, HC:HC + 1], 1.0,
                                            None, op0=Alu.max)
                    nc.vector.reciprocal(cnt[:], cnt[:])
                    pooled = wp.tile([G, HC], f32, tag="pooled")
                    nc.vector.tensor_scalar(pooled[:], pr[:, 0:HC],
                                            cnt[:, 0:1], None, op0=Alu.mult)
                    psl_full = ppre.tile([P, XC], f32, space="PSUM",
                                         tag="ppre")
                    psl = psl_full[0:G, 0:2]
                    for k in range(2):
                        tp = ptp.tile([P, P], f32, space="PSUM", tag="tp")
                        nc.tensor.transpose(tp[:, 0:G],
                                            pooled[:, k * P:(k + 1) * P],
                                            id_t[0:G, 0:G])
                        pt = wp.tile([P, G], f32, tag="pT")
                        nc.scalar.copy(pt[:], tp[:, 0:G])
                        nc.tensor.matmul(psl, lhsT=pt[:],
                                         rhs=wl_t[:, 2 * k:2 * k + 2],
                                         start=(k == 0), stop=(k == 1))
                    lg = wp.tile([G, 2], f32, tag="lg")
                    nc.vector.tensor_tensor(lg[:], psl, bl_t[:], op=Alu.add)
                    nc.sync.dma_start(logits[:, :], lg[:])
                else:
                    lg0 = wp.tile([G, 2], f32, tag="lg")
                    nc.vector.tensor_copy(lg0[:], bl_t[:])
                    nc.sync.dma_start(logits[:, :], lg0[:])

            for _rep in range(reps):
                phase_a()
                phases_cde()

    nc.compile()
    return nc


def kernel(**inputs):
    from concourse.bass_utils import run_bass_kernel_spmd

    nc, in_maps = prepare(inputs)
    res = run_bass_kernel_spmd(nc, in_maps, core_ids=list(range(NC_)))
    return res.results[0]["logits"]


def prepare(inputs):
    bf16 = _bf16()
    x = np.asarray(inputs["x"], np.float32)
    edge_index = np.asarray(inputs["edge_index"], np.int64)
    batch = np.asarray(inputs["batch"], np.int64)
    W1 = np.asarray(inputs["W1"], np.float32)
    W2 = np.asarray(inputs["W2"], np.float32)
    W_lin = np.asarray(inputs["W_lin"], np.float32)
    b1 = np.asarray(inputs["b1"], np.float32)
    b2 = np.asarray(inputs["b2"], np.float32)
    b_lin = np.asarray(inputs["b_lin"], np.float32)
    a_src1 = np.asarray(inputs["a_src1"], np.float32)
    a_dst1 = np.asarray(inputs["a_dst1"], np.float32)
    a_src2 = np.asarray(inputs["a_src2"], np.float32)
    a_dst2 = np.asarray(inputs["a_dst2"], np.float32)

    meta = _preprocess(edge_index, batch)
    nc = _build(meta)

    W1R, b1ext = _fold(W1, a_src1, a_dst1, b1)
    W2R, b2ext = _fold(W2, a_src2, a_dst2, b2)

    iota128 = np.tile(np.arange(P, dtype=np.float32), (P, 1))
    iota64 = np.tile(np.arange(G, dtype=np.float32), (P, 1))
    ident = np.eye(P, dtype=np.float32)
    identB = np.eye(P, dtype=np.float32).astype(bf16)
    wlin_p = np.concatenate([W_lin[0:P], W_lin[P:2 * P]], axis=1)

    in_maps = []
    for c in range(NC_):
        xs = np.zeros((P, SHP), np.float32)
        xs[:, :SH] = x[c * SH:(c + 1) * SH].T
        in_maps.append({
            "xT": xs,
            "W1R": W1R,
            "W2Ra": np.ascontiguousarray(W2R[0:P]),
            "W2Rb": np.ascontiguousarray(W2R[P:2 * P]),
            "b1e": np.tile(b1ext, (P, 1)),
            "b2e": np.tile(b2ext, (P, 1)),
            "Wlin": np.ascontiguousarray(wlin_p),
            "blin": np.tile(b_lin, (G, 1)),
            "iota128": iota128.astype(bf16),
            "iota64": iota64,
            "ident": ident, "identB": identB,
            "dstloc": np.ascontiguousarray(meta["dstloc"][c]).astype(bf16),
            "idxlo": np.ascontiguousarray(meta["ilo"][c]),
            "idxhi": np.ascontiguousarray(meta["ihi"][c]),
            "idxad": np.ascontiguousarray(meta["adtok"][c]),
            "batchloc": np.ascontiguousarray(meta["batchloc"][c]),
        })

    return nc, in_maps

